# revision 1
# baseline (speedup 1.0000x reference)
"""Causal multi-head attention (batch=4, seq=2048, d_model=768, 12 heads of 64)
on 8 TRN2 NeuronCores.

Sharding: core c handles batch c//2 and heads (c%2)*6 .. (c%2)*6+6
(data parallel over batch x tensor parallel over head halves).
Each core computes a partial output (its 6 heads' contribution, [2048, 768]);
the host sums the two half-partials per batch and adds biases.

Schedule (best-measured configuration; evidence from microbenches):
  - S matmuls (K=64) for the two heads of a pair are emitted adjacently with
    base partitions 0/64 -> row groups {0,1}/{2,3}, so the second weight load
    overlaps the first matmul (measured 237ns/mm vs 532 sequential).
  - exp runs as per-head 512-wide single-PSUM-bank ACT instructions with a
    4-buffer S pool and 2-group PV lookahead (multi-bank activation APs and
    shallow rotation are several times slower).
  - causal masking applied POST-exp on DVE (multiply the diagonal 128x128
    block by a 0/1 upper-triangular mask) - no PE mask matmuls.
  - Q/K/V projections are emitted as "filler" PE work interleaved between
    attention groups so the PE stays busy while ACT exponentiates.
PSUM banks: S pool 4 x [128,512] + z 2 x [65,512] + proj/O 2 x [128,512] = 8.
"""
import contextlib
import numpy as np

import concourse.bass as bass
import concourse.mybir as mybir
import concourse.tile as tile
from concourse import bacc
from concourse.bass_utils import run_bass_kernel_spmd
from concourse.masks import make_upper_triangular

BATCH, SEQ, DM, NH, DH = 4, 2048, 768, 12, 64
H = 6                 # heads per core
HD = H * DH           # 384
MC = DM // 128        # 6 m-chunks
NKT = SEQ // 128      # 16 k-tiles
NQC = SEQ // 512      # 4 q-chunks
F16 = mybir.dt.float16
F32 = mybir.dt.float32

_BUILD_CACHE = {}


def build(reps: int = 1, upto: str = "all"):
    key = (reps, upto)
    if key in _BUILD_CACHE:
        return _BUILD_CACHE[key]
    nc = bacc.Bacc("TRN2", target_bir_lowering=False, debug=False)
    xt_d = nc.dram_tensor("xt", [DM, SEQ], F16, kind="ExternalInput")
    wq_d = nc.dram_tensor("wq", [128, MC, HD], F16, kind="ExternalInput")
    wk_d = nc.dram_tensor("wk", [128, MC, HD], F16, kind="ExternalInput")
    wv_d = nc.dram_tensor("wv", [128, MC, HD], F16, kind="ExternalInput")
    wo_d = nc.dram_tensor("wo", [3, 128, DM], F16, kind="ExternalInput")
    bq_d = nc.dram_tensor("bq", [128, HD // 128], F32, kind="ExternalInput")
    bk_d = nc.dram_tensor("bk", [128, HD // 128], F32, kind="ExternalInput")
    o_d = nc.dram_tensor("out", [SEQ, DM], F16, kind="ExternalOutput")

    with tile.TileContext(nc) as tc:
        def body(_iv=None):
            with contextlib.ExitStack() as ctx:
                consts = ctx.enter_context(tc.tile_pool(name="consts", bufs=1))
                persist = ctx.enter_context(tc.tile_pool(name="persist", bufs=1))

                # ---- load inputs (already fp16 from host) ----
                # DMA issue is serial; order by lead-in need: wv + first x
                # chunks feed the V0-3 units, then wq/bq for the first Q
                # projection, then the rest.
                w16 = {}
                for name in ("wq", "wk", "wv"):
                    w16[name] = persist.tile([128, MC, HD], F16,
                                             name=f"{name}16")
                xt16 = [persist.tile([128, SEQ], F16, name=f"xt16_{c}")
                        for c in range(MC)]
                nc.sync.dma_start(out=w16["wv"], in_=wv_d.ap())
                for c in range(2):
                    nc.sync.dma_start(out=xt16[c],
                                      in_=xt_d.ap()[c * 128:(c + 1) * 128, :])
                nc.sync.dma_start(out=w16["wq"], in_=wq_d.ap())
                for c in range(2, MC):
                    nc.sync.dma_start(out=xt16[c],
                                      in_=xt_d.ap()[c * 128:(c + 1) * 128, :])
                nc.sync.dma_start(out=w16["wk"], in_=wk_d.ap())
                bq_s = consts.tile([128, HD // 128], F32)
                nc.sync.dma_start(out=bq_s, in_=bq_d.ap())
                bk_s = consts.tile([128, HD // 128], F32)
                nc.sync.dma_start(out=bk_s, in_=bk_d.ap())
                wo16 = []
                for j in range(3):
                    wot = persist.tile([128, DM], F16, name=f"wo16_{j}")
                    nc.sync.dma_start(out=wot, in_=wo_d.ap()[j])
                    wo16.append(wot)

                tri01 = consts.tile([128, 128], F16)
                make_upper_triangular(nc, tri01, val=1.0, diag=True)

                if upto == "load":
                    return

                qt = [persist.tile([128, SEQ], F16, name=f"qt{j}")
                      for j in range(3)]
                kt_ = [persist.tile([128, SEQ], F16, name=f"kt{j}")
                      for j in range(3)]
                vt = [persist.tile([128, H, DH + 1], F16, name=f"v{i}")
                      for i in range(NKT)]
                # pair-packed zT: rows 0..63 head 2j, 64..127 head 2j+1
                ztp = [persist.tile([128, SEQ], F16, name=f"ztp{j}")
                       for j in range(3)]

                s_ps = ctx.enter_context(
                    tc.tile_pool(name="s_ps", bufs=4, space="PSUM"))
                z_ps = ctx.enter_context(
                    tc.tile_pool(name="z_ps", bufs=2, space="PSUM"))
                pr_ps = ctx.enter_context(
                    tc.tile_pool(name="pr_ps", bufs=2, space="PSUM"))
                p_pool = ctx.enter_context(tc.tile_pool(name="p_pool", bufs=8))
                r_pool = ctx.enter_context(tc.tile_pool(name="r_pool", bufs=2))
                r0_pool = ctx.enter_context(tc.tile_pool(name="r0_pool", bufs=2))
                rb_pool = ctx.enter_context(tc.tile_pool(name="rb_pool", bufs=2))
                zo_pool = ctx.enter_context(tc.tile_pool(name="zo_pool", bufs=2))
                o_sb_pool = ctx.enter_context(tc.tile_pool(name="o_sb", bufs=2))

                # ---------- filler units (PE work to hide exp latency) ------
                def qk_unit(dst, w, b_s, j, qc):
                    def emit():
                        ps = pr_ps.tile([128, 512], F32, name="pp", tag="pr")
                        for c in range(MC):
                            nc.tensor.matmul(
                                ps,
                                w16[w][:, c, j * 128:(j + 1) * 128],
                                xt16[c][:, qc * 512:(qc + 1) * 512],
                                start=(c == 0), stop=(c == MC - 1))
                        nc.vector.tensor_scalar(
                            out=dst[j][:, qc * 512:(qc + 1) * 512],
                            in0=ps, scalar1=b_s[:, j:j + 1], scalar2=None,
                            op0=mybir.AluOpType.add)
                    return emit

                def v_unit(ktile):
                    def emit():
                        ps = pr_ps.tile([128, HD], F32, name="pv", tag="pr")
                        for c in range(MC):
                            nc.tensor.matmul(
                                ps,
                                xt16[c][:, ktile * 128:(ktile + 1) * 128],
                                w16["wv"][:, c, :],
                                start=(c == 0), stop=(c == MC - 1))
                        nc.vector.tensor_copy(
                            vt[ktile][:, :, 0:DH],
                            ps.rearrange("p (h d) -> p h d", h=H))
                        nc.vector.memset(vt[ktile][:, :, DH:DH + 1], 1.0)
                    return emit

                def o_unit(qtile):
                    def emit():
                        o_s = o_sb_pool.tile([128, DM], F16, name="os",
                                             tag="os")
                        for n0, w in [(0, 512), (512, 256)]:
                            o_t = pr_ps.tile([128, 512], F32, name="oo",
                                             tag="pr")
                            for j in range(3):
                                nc.tensor.matmul(
                                    o_t[:, 0:w],
                                    ztp[j][:, qtile * 128:(qtile + 1) * 128],
                                    wo16[j][:, n0:n0 + w],
                                    start=(j == 0), stop=(j == 2))
                            nc.vector.tensor_copy(o_s[:, n0:n0 + w],
                                                  o_t[:, 0:w])
                        nc.sync.dma_start(
                            out=o_d.ap()[qtile * 128:(qtile + 1) * 128, :],
                            in_=o_s)
                    return emit

                # ordered filler: V k-tiles and QK projections, just-in-time.
                # lead-in covers V0-3 and QK j0 qc0.
                filler = []
                for qc in range(1, NQC):
                    for t in range(4 * qc, 4 * qc + 4):
                        filler.append(("v", t, v_unit(t)))
                    filler.append(("qk", (0, qc), qk_unit(qt, "wq", bq_s, 0, qc)))
                    filler.append(("qk", (0, qc), qk_unit(kt_, "wk", bk_s, 0, qc)))
                for j in range(1, 3):
                    for qc in range(NQC):
                        filler.append(("qk", (j, qc),
                                       qk_unit(qt, "wq", bq_s, j, qc)))
                        filler.append(("qk", (j, qc),
                                       qk_unit(kt_, "wk", bk_s, j, qc)))
                fill_pos = [0]

                def pull_filler(n=1):
                    done = 0
                    while done < n and fill_pos[0] < len(filler):
                        filler[fill_pos[0]][2]()
                        fill_pos[0] += 1
                        done += 1

                def ensure_ready(j, qc):
                    # emit every filler unit needed before block (j, qc)
                    while fill_pos[0] < len(filler):
                        kind, key_, emit = filler[fill_pos[0]]
                        if kind == "v":
                            need = key_ <= 4 * qc + 3
                        else:
                            fj, fqc = key_
                            need = fj < j or (fj == j and fqc <= qc)
                        if not need:
                            break
                        emit()
                        fill_pos[0] += 1

                # ---------------- attention block for head pair j ------------
                def attn_block(j, qc):
                    qc0 = qc * 512
                    z_a = z_ps.tile([65, 512], F32, name="za", tag="z")
                    z_b = z_ps.tile([65, 512], F32, name="zb", tag="z")
                    strips = []
                    nkt_q = 4 * qc + 4

                    def flush(nmax):
                        while len(strips) > nmax:
                            ktile, p_a, p_b, cw = strips.pop(0)
                            nc.tensor.matmul(
                                z_a[:, cw:512], vt[ktile][:, 2 * j, :],
                                p_a[:, cw:512],
                                start=(ktile == 0), stop=(ktile == nkt_q - 1))
                            nc.tensor.matmul(
                                z_b[:, cw:512], vt[ktile][:, 2 * j + 1, :],
                                p_b[:, cw:512],
                                start=(ktile == 0), stop=(ktile == nkt_q - 1))

                    for ktile in range(nkt_q):
                        k0 = ktile * 128
                        diag = ktile >= 4 * qc
                        cw = max(0, k0 - qc0)
                        s_a = s_ps.tile([128, 512], F32, name="sa", tag="s")
                        s_b = s_ps.tile([128, 512], F32, name="sb", tag="s")
                        # adjacent row-tiled pair -> concurrent on PE
                        nc.tensor.matmul(
                            s_a[:, cw:512],
                            kt_[j][0:64, k0:k0 + 128],
                            qt[j][0:64, qc0 + cw:qc0 + 512],
                            start=True, stop=True)
                        nc.tensor.matmul(
                            s_b[:, cw:512],
                            kt_[j][64:128, k0:k0 + 128],
                            qt[j][64:128, qc0 + cw:qc0 + 512],
                            start=True, stop=True)
                        p_a = p_pool.tile([128, 512], F16, name="pa", tag="p")
                        p_b = p_pool.tile([128, 512], F16, name="pb", tag="p")
                        nc.scalar.activation(
                            p_a[:, cw:512], s_a[:, cw:512],
                            mybir.ActivationFunctionType.Exp)
                        nc.scalar.activation(
                            p_b[:, cw:512], s_b[:, cw:512],
                            mybir.ActivationFunctionType.Exp)
                        if diag:
                            nc.vector.tensor_tensor(
                                out=p_a[:, cw:cw + 128], in0=p_a[:, cw:cw + 128],
                                in1=tri01, op=mybir.AluOpType.mult)
                            nc.vector.tensor_tensor(
                                out=p_b[:, cw:cw + 128], in0=p_b[:, cw:cw + 128],
                                in1=tri01, op=mybir.AluOpType.mult)
                        strips.append((ktile, p_a, p_b, cw))
                        flush(2)
                        if ktile % 2 == 1:
                            pull_filler(1)
                    flush(0)

                    # normalize -> pair-packed ztp[j]
                    for hp, z_t in ((0, z_a), (64, z_b)):
                        r_t = r_pool.tile([65, 512], F32, name="r", tag="r")
                        nc.vector.reciprocal(out=r_t[64:65, :],
                                             in_=z_t[64:65, :])
                        r0_t = r0_pool.tile([1, 512], F32, name="r0", tag="r0")
                        nc.sync.dma_start(out=r0_t, in_=r_t[64:65, :])
                        rb_t = rb_pool.tile([64, 512], F32, name="rb", tag="rb")
                        nc.gpsimd.partition_broadcast(rb_t, r0_t)
                        if hp == 0:
                            nc.vector.tensor_tensor(
                                out=ztp[j][0:64, qc0:qc0 + 512],
                                in0=z_t[0:64, :], in1=rb_t,
                                op=mybir.AluOpType.mult)
                        else:
                            zo_t = zo_pool.tile([64, 512], F16, name="zo",
                                                tag="zo")
                            nc.vector.tensor_tensor(
                                out=zo_t, in0=z_t[0:64, :], in1=rb_t,
                                op=mybir.AluOpType.mult)
                            nc.sync.dma_start(
                                out=ztp[j][64:128, qc0:qc0 + 512], in_=zo_t)

                # ---------------- lead-in ----------------
                for t in range(4):
                    v_unit(t)()
                qk_unit(qt, "wq", bq_s, 0, 0)()
                qk_unit(kt_, "wk", bk_s, 0, 0)()

                if upto == "proj":
                    while fill_pos[0] < len(filler):
                        pull_filler(1)
                    return

                # ---------------- main: pair-major ----------------
                for j in range(3):
                    for qc in range(NQC):
                        ensure_ready(j, qc)
                        attn_block(j, qc)
                        if j == 2 and qc >= 1 and upto == "all":
                            # O-projection for q-chunk qc-1 (all pairs done)
                            for qtile in range(4 * (qc - 1), 4 * qc):
                                o_unit(qtile)()
                if upto != "all":
                    while fill_pos[0] < len(filler):
                        pull_filler(1)
                    return
                for qtile in range(12, 16):
                    o_unit(qtile)()

        if reps == 1:
            body()
        else:
            with tc.For_i(0, reps, 1) as _iv:
                body(_iv)

    nc.compile()
    _BUILD_CACHE[key] = nc
    return nc


def make_in_maps(normalized_resid_pre, W_Q, W_K, W_V, W_O, b_Q, b_K, b_V, b_O):
    scale = np.float32(1.0 / np.sqrt(DH))
    in_maps = []
    for core in range(8):
        b, h0 = core // 2, (core % 2) * H
        hs = slice(h0, h0 + H)
        in_maps.append({
            "xt": np.ascontiguousarray(
                normalized_resid_pre[b].T).astype(np.float16),
            "wq": (np.ascontiguousarray(
                W_Q[hs].transpose(1, 0, 2).reshape(MC, 128, HD)
                .transpose(1, 0, 2)) * scale).astype(np.float16),
            "wk": np.ascontiguousarray(
                W_K[hs].transpose(1, 0, 2).reshape(MC, 128, HD)
                .transpose(1, 0, 2)).astype(np.float16),
            "wv": np.ascontiguousarray(
                W_V[hs].transpose(1, 0, 2).reshape(MC, 128, HD)
                .transpose(1, 0, 2)).astype(np.float16),
            "wo": np.ascontiguousarray(
                W_O[hs].reshape(3, 128, DM)).astype(np.float16),
            "bq": np.ascontiguousarray(
                (np.asarray(b_Q)[hs].reshape(HD, 1).reshape(HD // 128, 128).T
                 * scale)).astype(np.float32),
            "bk": np.ascontiguousarray(
                np.asarray(b_K)[hs].reshape(HD // 128, 128).T).astype(
                np.float32),
        })
    return in_maps


def assemble(results, b_V, b_O, W_O):
    bv_wo = np.einsum("hd,hdm->m", b_V.astype(np.float64),
                      W_O.astype(np.float64)).astype(np.float32)
    out = np.empty((BATCH, SEQ, DM), dtype=np.float32)
    for b in range(BATCH):
        out[b] = (results[2 * b]["out"].astype(np.float32)
                  + results[2 * b + 1]["out"].astype(np.float32)
                  + b_O + bv_wo)
    return out


def kernel(normalized_resid_pre, W_Q, W_K, W_V, W_O, b_Q, b_K, b_V, b_O):
    nc = build(reps=1)
    in_maps = make_in_maps(normalized_resid_pre, W_Q, W_K, W_V, W_O,
                           b_Q, b_K, b_V, b_O)
    last_err = None
    for _attempt in range(3):
        try:
            res = run_bass_kernel_spmd(nc, in_maps, core_ids=list(range(8)))
            return assemble(res.results, b_V, b_O, W_O)
        except Exception as e:  # transient NRT/axon hiccups observed
            last_err = e
    raise last_err



# revision 14
# speedup vs baseline: 1.2293x; 1.2293x over previous
"""Causal multi-head attention (batch=4, seq=2048, d_model=768, 12 heads of 64)
on 8 TRN2 NeuronCores.

Sharding: core c handles batch c//2 and heads (c%2)*6 .. (c%2)*6+6
(data parallel over batch x tensor parallel over head halves).
Each core computes a partial output (its 6 heads' contribution, [2048, 768]);
the host sums the two half-partials per batch and adds biases.

Schedule (best-measured configuration; evidence from microbenches):
  - S matmuls (K=64) for the two heads of a pair are emitted adjacently with
    base partitions 0/64 -> row groups {0,1}/{2,3}, so the second weight load
    overlaps the first matmul (measured 237ns/mm vs 532 sequential).
  - softmax reciprocal via reciprocal_approx_fast (~762ns vs ~3577ns for
    nc.vector.reciprocal, which dominated DVE time at 48 calls/iteration).
  - exp runs as per-head 512-wide single-PSUM-bank ACT instructions with a
    4-buffer S pool and 2-group PV lookahead (multi-bank activation APs and
    shallow rotation are several times slower).
  - causal masking applied POST-exp on DVE (multiply the diagonal 128x128
    block by a 0/1 upper-triangular mask) - no PE mask matmuls.
  - Q/K/V projections are emitted as "filler" PE work interleaved between
    attention groups so the PE stays busy while ACT exponentiates.
PSUM banks: S pool 4 x [128,512] + z 2 x [65,512] + proj/O 2 x [128,512] = 8.
"""
import contextlib
import numpy as np

import concourse.bass as bass
import concourse.mybir as mybir
import concourse.tile as tile
from concourse import bacc
from concourse.bass_utils import run_bass_kernel_spmd
from concourse.masks import make_upper_triangular

BATCH, SEQ, DM, NH, DH = 4, 2048, 768, 12, 64
H = 6                 # heads per core
HD = H * DH           # 384
MC = DM // 128        # 6 m-chunks
NKT = SEQ // 128      # 16 k-tiles
NQC = SEQ // 512      # 4 q-chunks
F16 = mybir.dt.float16
F32 = mybir.dt.float32

_BUILD_CACHE = {}


def build(reps: int = 1, upto: str = "all"):
    key = (reps, upto)
    if key in _BUILD_CACHE:
        return _BUILD_CACHE[key]
    nc = bacc.Bacc("TRN2", target_bir_lowering=False, debug=False)
    xt_d = nc.dram_tensor("xt", [DM, SEQ], F16, kind="ExternalInput")
    wq_d = nc.dram_tensor("wq", [128, MC, HD], F16, kind="ExternalInput")
    wk_d = nc.dram_tensor("wk", [128, MC, HD], F16, kind="ExternalInput")
    wv_d = nc.dram_tensor("wv", [128, MC, HD], F16, kind="ExternalInput")
    wo_d = nc.dram_tensor("wo", [3, 128, DM], F16, kind="ExternalInput")
    bq_d = nc.dram_tensor("bq", [128, HD // 128], F32, kind="ExternalInput")
    bk_d = nc.dram_tensor("bk", [128, HD // 128], F32, kind="ExternalInput")
    o_d = nc.dram_tensor("out", [SEQ, DM], F16, kind="ExternalOutput")

    with tile.TileContext(nc) as tc:
        def body(_iv=None):
            with contextlib.ExitStack() as ctx:
                consts = ctx.enter_context(tc.tile_pool(name="consts", bufs=1))
                persist = ctx.enter_context(tc.tile_pool(name="persist", bufs=1))

                # ---- load inputs (already fp16 from host) ----
                # DMA issue is serial; order by lead-in need: wv + first x
                # chunks feed the V0-3 units, then wq/bq for the first Q
                # projection, then the rest.
                w16 = {}
                for name in ("wq", "wk", "wv"):
                    w16[name] = persist.tile([128, MC, HD], F16,
                                             name=f"{name}16")
                xt16 = [persist.tile([128, SEQ], F16, name=f"xt16_{c}")
                        for c in range(MC)]
                nc.sync.dma_start(out=w16["wv"], in_=wv_d.ap())
                for c in range(2):
                    nc.sync.dma_start(out=xt16[c],
                                      in_=xt_d.ap()[c * 128:(c + 1) * 128, :])
                nc.sync.dma_start(out=w16["wq"], in_=wq_d.ap())
                for c in range(2, MC):
                    nc.sync.dma_start(out=xt16[c],
                                      in_=xt_d.ap()[c * 128:(c + 1) * 128, :])
                nc.sync.dma_start(out=w16["wk"], in_=wk_d.ap())
                bq_s = consts.tile([128, HD // 128], F32)
                nc.sync.dma_start(out=bq_s, in_=bq_d.ap())
                bk_s = consts.tile([128, HD // 128], F32)
                nc.sync.dma_start(out=bk_s, in_=bk_d.ap())
                wo16 = []
                for j in range(3):
                    wot = persist.tile([128, DM], F16, name=f"wo16_{j}")
                    nc.sync.dma_start(out=wot, in_=wo_d.ap()[j])
                    wo16.append(wot)

                tri01 = consts.tile([128, 128], F16)
                make_upper_triangular(nc, tri01, val=1.0, diag=True)

                if upto == "load":
                    return

                qt = [persist.tile([128, SEQ], F16, name=f"qt{j}")
                      for j in range(3)]
                kt_ = [persist.tile([128, SEQ], F16, name=f"kt{j}")
                      for j in range(3)]
                vt = [persist.tile([128, H, DH + 1], F16, name=f"v{i}")
                      for i in range(NKT)]
                # pair-packed zT: rows 0..63 head 2j, 64..127 head 2j+1
                ztp = [persist.tile([128, SEQ], F16, name=f"ztp{j}")
                       for j in range(3)]

                s_ps = ctx.enter_context(
                    tc.tile_pool(name="s_ps", bufs=4, space="PSUM"))
                z_ps = ctx.enter_context(
                    tc.tile_pool(name="z_ps", bufs=2, space="PSUM"))
                pr_ps = ctx.enter_context(
                    tc.tile_pool(name="pr_ps", bufs=2, space="PSUM"))
                p_pool = ctx.enter_context(tc.tile_pool(name="p_pool", bufs=8))
                r_pool = ctx.enter_context(tc.tile_pool(name="r_pool", bufs=2))
                r0_pool = ctx.enter_context(tc.tile_pool(name="r0_pool", bufs=2))
                rb_pool = ctx.enter_context(tc.tile_pool(name="rb_pool", bufs=2))
                zo_pool = ctx.enter_context(tc.tile_pool(name="zo_pool", bufs=2))
                o_sb_pool = ctx.enter_context(tc.tile_pool(name="o_sb", bufs=2))

                # ---------- filler units (PE work to hide exp latency) ------
                def qk_unit(dst, w, b_s, j, qc):
                    def emit():
                        ps = pr_ps.tile([128, 512], F32, name="pp", tag="pr")
                        for c in range(MC):
                            nc.tensor.matmul(
                                ps,
                                w16[w][:, c, j * 128:(j + 1) * 128],
                                xt16[c][:, qc * 512:(qc + 1) * 512],
                                start=(c == 0), stop=(c == MC - 1))
                        nc.vector.tensor_scalar(
                            out=dst[j][:, qc * 512:(qc + 1) * 512],
                            in0=ps, scalar1=b_s[:, j:j + 1], scalar2=None,
                            op0=mybir.AluOpType.add)
                    return emit

                def v_unit(ktile):
                    def emit():
                        ps = pr_ps.tile([128, HD], F32, name="pv", tag="pr")
                        for c in range(MC):
                            nc.tensor.matmul(
                                ps,
                                xt16[c][:, ktile * 128:(ktile + 1) * 128],
                                w16["wv"][:, c, :],
                                start=(c == 0), stop=(c == MC - 1))
                        nc.vector.tensor_copy(
                            vt[ktile][:, :, 0:DH],
                            ps.rearrange("p (h d) -> p h d", h=H))
                        nc.vector.memset(vt[ktile][:, :, DH:DH + 1], 1.0)
                    return emit

                def o_unit(qtile):
                    def emit():
                        o_s = o_sb_pool.tile([128, DM], F16, name="os",
                                             tag="os")
                        for n0, w in [(0, 512), (512, 256)]:
                            o_t = pr_ps.tile([128, 512], F32, name="oo",
                                             tag="pr")
                            for j in range(3):
                                nc.tensor.matmul(
                                    o_t[:, 0:w],
                                    ztp[j][:, qtile * 128:(qtile + 1) * 128],
                                    wo16[j][:, n0:n0 + w],
                                    start=(j == 0), stop=(j == 2))
                            nc.vector.tensor_copy(o_s[:, n0:n0 + w],
                                                  o_t[:, 0:w])
                        nc.sync.dma_start(
                            out=o_d.ap()[qtile * 128:(qtile + 1) * 128, :],
                            in_=o_s)
                    return emit

                # ordered filler: V k-tiles and QK projections, just-in-time.
                # lead-in covers V0-3 and QK j0 qc0.
                filler = []
                for qc in range(1, NQC):
                    for t in range(4 * qc, 4 * qc + 4):
                        filler.append(("v", t, v_unit(t)))
                    filler.append(("qk", (0, qc), qk_unit(qt, "wq", bq_s, 0, qc)))
                    filler.append(("qk", (0, qc), qk_unit(kt_, "wk", bk_s, 0, qc)))
                for j in range(1, 3):
                    for qc in range(NQC):
                        filler.append(("qk", (j, qc),
                                       qk_unit(qt, "wq", bq_s, j, qc)))
                        filler.append(("qk", (j, qc),
                                       qk_unit(kt_, "wk", bk_s, j, qc)))
                fill_pos = [0]

                def pull_filler(n=1):
                    done = 0
                    while done < n and fill_pos[0] < len(filler):
                        filler[fill_pos[0]][2]()
                        fill_pos[0] += 1
                        done += 1

                def ensure_ready(j, qc):
                    # emit every filler unit needed before block (j, qc)
                    while fill_pos[0] < len(filler):
                        kind, key_, emit = filler[fill_pos[0]]
                        if kind == "v":
                            need = key_ <= 4 * qc + 3
                        else:
                            fj, fqc = key_
                            need = fj < j or (fj == j and fqc <= qc)
                        if not need:
                            break
                        emit()
                        fill_pos[0] += 1

                # ---------------- attention block for head pair j ------------
                def attn_block(j, qc):
                    qc0 = qc * 512
                    z_a = z_ps.tile([65, 512], F32, name="za", tag="z")
                    z_b = z_ps.tile([65, 512], F32, name="zb", tag="z")
                    strips = []
                    nkt_q = 4 * qc + 4

                    def flush(nmax):
                        while len(strips) > nmax:
                            ktile, p_a, p_b, cw = strips.pop(0)
                            nc.tensor.matmul(
                                z_a[:, cw:512], vt[ktile][:, 2 * j, :],
                                p_a[:, cw:512],
                                start=(ktile == 0), stop=(ktile == nkt_q - 1))
                            nc.tensor.matmul(
                                z_b[:, cw:512], vt[ktile][:, 2 * j + 1, :],
                                p_b[:, cw:512],
                                start=(ktile == 0), stop=(ktile == nkt_q - 1))

                    for ktile in range(nkt_q):
                        k0 = ktile * 128
                        diag = ktile >= 4 * qc
                        cw = max(0, k0 - qc0)
                        s_a = s_ps.tile([128, 512], F32, name="sa", tag="s")
                        s_b = s_ps.tile([128, 512], F32, name="sb", tag="s")
                        # adjacent row-tiled pair -> concurrent on PE
                        nc.tensor.matmul(
                            s_a[:, cw:512],
                            kt_[j][0:64, k0:k0 + 128],
                            qt[j][0:64, qc0 + cw:qc0 + 512],
                            start=True, stop=True)
                        nc.tensor.matmul(
                            s_b[:, cw:512],
                            kt_[j][64:128, k0:k0 + 128],
                            qt[j][64:128, qc0 + cw:qc0 + 512],
                            start=True, stop=True)
                        p_a = p_pool.tile([128, 512], F16, name="pa", tag="p")
                        p_b = p_pool.tile([128, 512], F16, name="pb", tag="p")
                        nc.scalar.activation(
                            p_a[:, cw:512], s_a[:, cw:512],
                            mybir.ActivationFunctionType.Exp)
                        nc.scalar.activation(
                            p_b[:, cw:512], s_b[:, cw:512],
                            mybir.ActivationFunctionType.Exp)
                        if diag:
                            nc.vector.tensor_tensor(
                                out=p_a[:, cw:cw + 128], in0=p_a[:, cw:cw + 128],
                                in1=tri01, op=mybir.AluOpType.mult)
                            nc.vector.tensor_tensor(
                                out=p_b[:, cw:cw + 128], in0=p_b[:, cw:cw + 128],
                                in1=tri01, op=mybir.AluOpType.mult)
                        strips.append((ktile, p_a, p_b, cw))
                        flush(2)
                        if ktile % 2 == 1:
                            pull_filler(1)
                    flush(0)

                    # normalize -> pair-packed ztp[j]
                    for hp, z_t in ((0, z_a), (64, z_b)):
                        # copy raw denominator row to SBUF (plain DVE copy -
                        # reciprocal_approx_fast can't read PSUM), broadcast
                        # it, then approx-reciprocal on the SBUF broadcast.
                        r_t = r_pool.tile([65, 512], F32, name="r", tag="r")
                        nc.vector.tensor_copy(r_t[64:65, :], z_t[64:65, :])
                        r0_t = r0_pool.tile([1, 512], F32, name="r0", tag="r0")
                        nc.sync.dma_start(out=r0_t, in_=r_t[64:65, :])
                        rb_t = rb_pool.tile([64, 512], F32, name="rb", tag="rb")
                        nc.gpsimd.partition_broadcast(rb_t, r0_t)
                        nc.vector.reciprocal_approx_fast(out=rb_t, in_=rb_t)
                        if hp == 0:
                            nc.vector.tensor_tensor(
                                out=ztp[j][0:64, qc0:qc0 + 512],
                                in0=z_t[0:64, :], in1=rb_t,
                                op=mybir.AluOpType.mult)
                        else:
                            zo_t = zo_pool.tile([64, 512], F16, name="zo",
                                                tag="zo")
                            nc.vector.tensor_tensor(
                                out=zo_t, in0=z_t[0:64, :], in1=rb_t,
                                op=mybir.AluOpType.mult)
                            nc.sync.dma_start(
                                out=ztp[j][64:128, qc0:qc0 + 512], in_=zo_t)

                # ---------------- lead-in ----------------
                for t in range(4):
                    v_unit(t)()
                qk_unit(qt, "wq", bq_s, 0, 0)()
                qk_unit(kt_, "wk", bk_s, 0, 0)()

                if upto == "proj":
                    while fill_pos[0] < len(filler):
                        pull_filler(1)
                    return

                # ---------------- main: pair-major ----------------
                for j in range(3):
                    for qc in range(NQC):
                        ensure_ready(j, qc)
                        attn_block(j, qc)
                        if j == 2 and qc >= 1 and upto == "all":
                            # O-projection for q-chunk qc-1 (all pairs done)
                            for qtile in range(4 * (qc - 1), 4 * qc):
                                o_unit(qtile)()
                if upto != "all":
                    while fill_pos[0] < len(filler):
                        pull_filler(1)
                    return
                for qtile in range(12, 16):
                    o_unit(qtile)()

        if reps == 1:
            body()
        else:
            with tc.For_i(0, reps, 1) as _iv:
                body(_iv)

    nc.compile()
    _BUILD_CACHE[key] = nc
    return nc


def make_in_maps(normalized_resid_pre, W_Q, W_K, W_V, W_O, b_Q, b_K, b_V, b_O):
    scale = np.float32(1.0 / np.sqrt(DH))
    in_maps = []
    for core in range(8):
        b, h0 = core // 2, (core % 2) * H
        hs = slice(h0, h0 + H)
        in_maps.append({
            "xt": np.ascontiguousarray(
                normalized_resid_pre[b].T).astype(np.float16),
            "wq": (np.ascontiguousarray(
                W_Q[hs].transpose(1, 0, 2).reshape(MC, 128, HD)
                .transpose(1, 0, 2)) * scale).astype(np.float16),
            "wk": np.ascontiguousarray(
                W_K[hs].transpose(1, 0, 2).reshape(MC, 128, HD)
                .transpose(1, 0, 2)).astype(np.float16),
            "wv": np.ascontiguousarray(
                W_V[hs].transpose(1, 0, 2).reshape(MC, 128, HD)
                .transpose(1, 0, 2)).astype(np.float16),
            "wo": np.ascontiguousarray(
                W_O[hs].reshape(3, 128, DM)).astype(np.float16),
            "bq": np.ascontiguousarray(
                (np.asarray(b_Q)[hs].reshape(HD, 1).reshape(HD // 128, 128).T
                 * scale)).astype(np.float32),
            "bk": np.ascontiguousarray(
                np.asarray(b_K)[hs].reshape(HD // 128, 128).T).astype(
                np.float32),
        })
    return in_maps


def assemble(results, b_V, b_O, W_O):
    bv_wo = np.einsum("hd,hdm->m", b_V.astype(np.float64),
                      W_O.astype(np.float64)).astype(np.float32)
    out = np.empty((BATCH, SEQ, DM), dtype=np.float32)
    for b in range(BATCH):
        out[b] = (results[2 * b]["out"].astype(np.float32)
                  + results[2 * b + 1]["out"].astype(np.float32)
                  + b_O + bv_wo)
    return out


def kernel(normalized_resid_pre, W_Q, W_K, W_V, W_O, b_Q, b_K, b_V, b_O):
    nc = build(reps=1)
    in_maps = make_in_maps(normalized_resid_pre, W_Q, W_K, W_V, W_O,
                           b_Q, b_K, b_V, b_O)
    last_err = None
    for _attempt in range(3):
        try:
            res = run_bass_kernel_spmd(nc, in_maps, core_ids=list(range(8)))
            return assemble(res.results, b_V, b_O, W_O)
        except Exception as e:  # transient NRT/axon hiccups observed
            last_err = e
    raise last_err

